# revision 11
# baseline (speedup 1.0000x reference)
"""Trainium2 Bass kernel for nn_Attention_7945689497706.

Reference math (per batch element b of 16):
  xn = RMSNorm(x) = x / ||x||_c * (gamma+1) * sqrt(512)        x: [512, 32*32]
  qkv = w_qkv @ xn ; split q,k,v [512, 1024] each; heads of 64
  q *= 64^-0.5 ; k,v get 4 mem_kv rows prepended (shared across batch)
  out = softmax(q k^T) v per head -> w_out @ out

Distribution: data-parallel over batch, 2 batch elements per core, weights
replicated. No collectives.

Kernel layout strategy (per core):
  - x kept [c, n] (c on partitions, 4 tiles of 128). Per-pixel inverse norm is
    computed with a ones-matmul (partition reduction) broadcast to all 128
    partitions; gamma folding happens on the weights (device-side, once).
  - q^T,k^T computed as [o, n] (fp32r matmuls); v as [n, o] (pixel-partition).
  - Attention runs transposed: sim_T[j, i] = k_T^T q_T with j (kv index) on
    psum partitions, i free. K is zero-padded to 128 (K=64 matmuls are 2x
    slower on PE and row-tiling is pathological). exp on ACT at [128, 1024]
    grain. av matmul: lhsT = [v | 1] (j, 65) bf16 -> out_T [65, i]; row 64
    accumulates the softmax denominator. Normalization via DVE reciprocal +
    DMA partition-broadcast + DVE multiply into attnout [c', n] fp32r.
  - mem_kv + padding live in a 9th j-chunk: k columns are zero there except
    the 4 mem rows, and vext rows beyond the mem rows are zero, so the padded
    exp(0)=1 entries contribute nothing to either the numerator or denominator.
"""

import numpy as np

import concourse.bass as bass
import concourse.mybir as mybir
import concourse.tile as tile
from concourse import bacc
from concourse.bass_utils import run_bass_kernel_spmd

F32 = mybir.dt.float32
F32R = mybir.dt.float32r
BF16 = mybir.dt.bfloat16
AF = mybir.ActivationFunctionType

NCORES = 8
B = 16
C = 512
N = 1024          # pixels = 32*32
HEADS = 8
DH = 64
NMEM = 4
PB = B // NCORES  # batch elements per core
CT = C // 128     # channel partition-tiles
JC = 9            # j chunks: 8 pixel chunks + 1 (mem + zero pad)
VW = HEADS * (DH + 1)  # vext width: per head [v | ones] = 65


def _build():
    nc = bacc.Bacc()
    x_ext = nc.declare_dram_parameter("x", [PB, C, N], F32, isOutput=False)
    wqkvt_ext = nc.declare_dram_parameter("wqkvt", [C, 3 * C], F32, isOutput=False)
    wot_ext = nc.declare_dram_parameter("wot", [C, C], F32, isOutput=False)
    gammat_ext = nc.declare_dram_parameter("gammat", [128, CT], F32, isOutput=False)
    memk_ext = nc.declare_dram_parameter("memk", [128, HEADS, 128], F32, isOutput=False)
    memv_ext = nc.declare_dram_parameter("memv", [128, VW], F32, isOutput=False)
    out_ext = nc.declare_dram_parameter("out", [PB, C, N], F32, isOutput=True)

    with tile.TileContext(nc) as tc:
        with (
            tc.tile_pool(name="const", bufs=1) as const,
            tc.tile_pool(name="wstage", bufs=1) as wstage,
            tc.tile_pool(name="xp", bufs=2) as xp,
            tc.tile_pool(name="data", bufs=1) as data,
            tc.tile_pool(name="qp", bufs=2) as qp,
            tc.tile_pool(name="pp", bufs=4) as pp,
            tc.tile_pool(name="avs", bufs=2) as avsp,
            tc.tile_pool(name="rp", bufs=2) as rp,
            tc.tile_pool(name="ob", bufs=2) as obp,
            tc.tile_pool(name="qkv_ps", bufs=2, space="PSUM") as qkv_ps,
            tc.tile_pool(name="sim_ps", bufs=2, space="PSUM") as sim_ps,
            tc.tile_pool(name="av_ps", bufs=2, space="PSUM") as av_ps,
        ):
            # ---------------- x loads first (don't queue behind weight DMAs) ----
            xraws = []
            for bb in range(PB):
                xr = xp.tile([128, CT, N], F32, tag="xraw")
                for t in range(CT):
                    eng = nc.sync if t % 2 == 0 else nc.scalar
                    eng.dma_start(out=xr[:, t, :], in_=x_ext[bb, t * 128:(t + 1) * 128, :])
                xraws.append(xr)

            # ---------------- per-core constants ----------------
            wqkv = const.tile([128, CT, 3 * C], F32R, tag="wqkv")
            wo = const.tile([128, CT, C], F32R, tag="wo")
            g1 = const.tile([128, CT], F32, tag="g1")
            g1q = const.tile([128, CT], F32, tag="g1q")
            ones128 = const.tile([128, 128], BF16, tag="ones128")
            ones1 = const.tile([128, 64], F32R, tag="ones1")
            kTp = const.tile([128, HEADS, 128 * JC], BF16, tag="kTp")
            vext = const.tile([128, JC, VW], BF16, tag="vext")

            gsb = const.tile([128, CT], F32, tag="gsb")
            nc.gpsimd.dma_start(out=gsb, in_=gammat_ext[:, :])
            nc.scalar.activation(out=g1, in_=gsb, func=AF.Copy, bias=1.0)
            nc.scalar.activation(out=g1q, in_=gsb, func=AF.Copy, bias=1.0, scale=1.0)
            nc.scalar.mul(out=g1q, in_=g1q, mul=DH ** -0.5)

            nc.vector.memset(ones128, 1.0)
            nc.vector.memset(ones1.bitcast(F32), 1.0)
            nc.vector.memset(kTp, 0.0)
            nc.vector.memset(vext, 0.0)

            # weights: DMA f32 staging, scale by (gamma+1) [and 1/sqrt(dh) for q]
            for t in range(CT):
                ws = wstage.tile([128, 3 * C], F32, tag="ws")
                nc.gpsimd.dma_start(out=ws, in_=wqkvt_ext[t * 128:(t + 1) * 128, :])
                nc.scalar.activation(out=wqkv[:, t, 0:C], in_=ws[:, 0:C],
                                     func=AF.Copy, scale=g1q[:, t:t + 1])
                nc.scalar.activation(out=wqkv[:, t, C:3 * C], in_=ws[:, C:3 * C],
                                     func=AF.Copy, scale=g1[:, t:t + 1])
            for t in range(CT):
                ws = wstage.tile([128, 3 * C], F32, tag="ws")
                nc.gpsimd.dma_start(out=ws[:, 0:C], in_=wot_ext[t * 128:(t + 1) * 128, :])
                nc.vector.tensor_copy(out=wo[:, t, :], in_=ws[:, 0:C])

            # mem_kv constants -> bf16 tiles (9th j-chunk)
            ws = wstage.tile([128, 3 * C], F32, tag="ws")
            nc.gpsimd.dma_start(out=ws[:, 0:HEADS * 128],
                              in_=memk_ext[:, :, :].rearrange("p h c -> p (h c)"))
            nc.vector.tensor_copy(
                out=kTp[:, :, 8 * 128:9 * 128],
                in_=ws[:, 0:HEADS * 128].rearrange("p (h c) -> p h c", c=128))
            ws2 = wstage.tile([128, 3 * C], F32, tag="ws")
            nc.gpsimd.dma_start(out=ws2[:, 0:VW], in_=memv_ext[:, :])
            nc.vector.tensor_copy(out=vext[:, 8, :], in_=ws2[:, 0:VW])
            # ones column for the pixel j-chunks (col 64 of each head slot)
            ones_cols = vext[:, 0:8, :].rearrange("p j (h c) -> p j h c", c=DH + 1)[:, :, :, DH:DH + 1]
            nc.vector.memset(ones_cols, 1.0)

            # ---------------- per-batch-element pipeline ----------------
            for bb in range(PB):
                # RMSNorm scale (per-pixel, broadcast on partitions)
                xraw = xraws[bb]
                xsq = data.tile([128, CT, N], BF16, tag="xsq")
                for t in range(CT):
                    nc.vector.tensor_mul(out=xsq[:, t, :], in0=xraw[:, t, :], in1=xraw[:, t, :])
                ss = sim_ps.tile([128, N], F32, tag="sim")
                for h2 in range(2):
                    for t in range(CT):
                        nc.tensor.matmul(ss[:, h2 * 512:(h2 + 1) * 512], ones128,
                                         xsq[:, t, h2 * 512:(h2 + 1) * 512],
                                         start=(t == 0), stop=(t == CT - 1))
                sroot = data.tile([128, N], F32, tag="sroot")
                nc.scalar.activation(out=sroot, in_=ss, func=AF.Sqrt, scale=1.0 / C)
                snorm = data.tile([128, N], F32, tag="snorm")
                nc.vector.reciprocal_approx_fast(out=snorm, in_=sroot)
                xn = data.tile([128, CT, N], F32R, tag="xn")
                for t in range(CT):
                    nc.vector.tensor_mul(out=xn[:, t, :], in0=xraw[:, t, :], in1=snorm)

                # q^T, k^T projections: [o, n] (fp32r)
                qT = qp.tile([128, CT, N], BF16, tag="qT")
                for mc in range(8):  # o-chunks: 0-3 q, 4-7 k
                    for h2 in range(2):
                        ps = qkv_ps.tile([128, 512], F32, tag="q")
                        for t in range(CT):
                            nc.tensor.matmul(ps, wqkv[:, t, mc * 128:(mc + 1) * 128],
                                             xn[:, t, h2 * 512:(h2 + 1) * 512],
                                             start=(t == 0), stop=(t == CT - 1))
                        if mc < 4:
                            nc.vector.tensor_copy(out=qT[:, mc, h2 * 512:(h2 + 1) * 512], in_=ps)
                        else:
                            h0, h1 = 2 * (mc - 4), 2 * (mc - 4) + 1
                            nc.vector.tensor_copy(
                                out=kTp[0:64, h0, h2 * 512:(h2 + 1) * 512], in_=ps[0:64, :])
                            nc.vector.tensor_copy(
                                out=kTp[64:128, h1, h2 * 512:(h2 + 1) * 512], in_=ps[64:128, :])

                # v projection: [n, o_v] (pixel partitions) -> vext (bf16, strided)
                for ic in range(8):
                    ps = qkv_ps.tile([128, 512], F32, tag="q")
                    for t in range(CT):
                        nc.tensor.matmul(ps, xn[:, t, ic * 128:(ic + 1) * 128],
                                         wqkv[:, t, 2 * C:3 * C],
                                         start=(t == 0), stop=(t == CT - 1))
                    ps_h = ps[:, :].rearrange("p (h c) -> p h c", c=DH)
                    vdst = vext[:, ic, :].rearrange("p (h c) -> p h c", c=DH + 1)[:, :, 0:DH]
                    nc.vector.tensor_copy(out=vdst, in_=ps_h)

                # attention per head
                attn = data.tile([128, CT, N], F32R, tag="attn")
                for h in range(HEADS):
                    av0 = av_ps.tile([65, 512], F32, tag="av")
                    av1 = av_ps.tile([65, 512], F32, tag="av")
                    avt = (av0, av1)
                    for jc in range(JC):
                        st = sim_ps.tile([128, N], F32, tag="sim")
                        for h2 in range(2):
                            nc.tensor.matmul(st[:, h2 * 512:(h2 + 1) * 512],
                                             kTp[:, h, jc * 128:(jc + 1) * 128],
                                             qT[:, h // 2, h2 * 512:(h2 + 1) * 512],
                                             start=True, stop=True)
                        p = pp.tile([128, N], BF16, tag="p")
                        nc.scalar.activation(out=p, in_=st, func=AF.Exp)
                        for h2 in range(2):
                            nc.tensor.matmul(avt[h2], vext[:, jc, h * (DH + 1):(h + 1) * (DH + 1)],
                                             p[:, h2 * 512:(h2 + 1) * 512],
                                             start=(jc == 0), stop=(jc == JC - 1))
                    for h2 in range(2):
                        avb = avsp.tile([65, 512], F32R, tag="avs")
                        nc.vector.tensor_copy(out=avb, in_=avt[h2])
                        bc = av_ps.tile([64, 512], F32, tag="av")
                        nc.tensor.matmul(bc, ones1[64:65, :], avb[64:65, :], start=True, stop=True)
                        rcp = rp.tile([64, 512], F32, tag="rcp")
                        nc.vector.reciprocal_approx_fast(out=rcp, in_=bc)
                        nc.vector.tensor_mul(
                            out=attn[64 * (h % 2):64 * (h % 2) + 64, h // 2,
                                     h2 * 512:(h2 + 1) * 512],
                            in0=avb[0:64, :].bitcast(F32), in1=rcp)

                # output projection [o, n] fp32r and store
                for mc in range(CT):
                    ob = obp.tile([128, N], F32, tag="ob")
                    for h2 in range(2):
                        ps = qkv_ps.tile([128, 512], F32, tag="q")
                        for t in range(CT):
                            nc.tensor.matmul(ps, wo[:, t, mc * 128:(mc + 1) * 128],
                                             attn[:, t, h2 * 512:(h2 + 1) * 512],
                                             start=(t == 0), stop=(t == CT - 1))
                        nc.vector.tensor_copy(out=ob[:, h2 * 512:(h2 + 1) * 512], in_=ps)
                    nc.sync.dma_start(out=out_ext[bb, mc * 128:(mc + 1) * 128, :], in_=ob)
    nc.compile()
    return nc


_NC_CACHE = []


def kernel(x, gamma, mem_kv, w_qkv, w_out, _trace=False):
    x = np.asarray(x, dtype=np.float32)
    gamma = np.asarray(gamma, dtype=np.float32)
    mem_kv = np.asarray(mem_kv, dtype=np.float32)
    w_qkv = np.asarray(w_qkv, dtype=np.float32)
    w_out = np.asarray(w_out, dtype=np.float32)

    b, c, hh, ww = x.shape
    n = hh * ww
    xs = x.reshape(b, c, n)

    wqkvt = np.ascontiguousarray(w_qkv.T)          # [c, 3c]
    wot = np.ascontiguousarray(w_out.T)            # [c, c]
    gammat = np.ascontiguousarray(gamma.reshape(CT, 128).T)  # [128, CT]

    memk = np.zeros((128, HEADS, 128), np.float32)
    memv = np.zeros((128, VW), np.float32)
    for h in range(HEADS):
        r0 = 64 * (h % 2)
        memk[r0:r0 + DH, h, 0:NMEM] = mem_kv[0, h].T      # [dh, nmem]
        memv[0:NMEM, h * (DH + 1):h * (DH + 1) + DH] = mem_kv[1, h]
        memv[0:NMEM, h * (DH + 1) + DH] = 1.0

    if not _NC_CACHE:
        _NC_CACHE.append(_build())
    nc = _NC_CACHE[0]

    in_maps = []
    for core in range(NCORES):
        in_maps.append({
            "x": np.ascontiguousarray(xs[core * PB:(core + 1) * PB]),
            "wqkvt": wqkvt,
            "wot": wot,
            "gammat": gammat,
            "memk": memk,
            "memv": memv,
        })
    res = run_bass_kernel_spmd(nc, in_maps, core_ids=list(range(NCORES)), trace=_trace)
    out = np.concatenate([res.results[core]["out"] for core in range(NCORES)], axis=0)
    kernel.last_result = res
    return out.reshape(b, c, hh, ww)


# revision 12
# speedup vs baseline: 1.0012x; 1.0012x over previous
"""Trainium2 Bass kernel for nn_Attention_7945689497706.

Reference math (per batch element b of 16):
  xn = RMSNorm(x) = x / ||x||_c * (gamma+1) * sqrt(512)        x: [512, 32*32]
  qkv = w_qkv @ xn ; split q,k,v [512, 1024] each; heads of 64
  q *= 64^-0.5 ; k,v get 4 mem_kv rows prepended (shared across batch)
  out = softmax(q k^T) v per head -> w_out @ out

Distribution: data-parallel over batch, 2 batch elements per core, weights
replicated. No collectives.

Kernel layout strategy (per core):
  - x kept [c, n] (c on partitions, 4 tiles of 128). Per-pixel inverse norm is
    computed with a ones-matmul (partition reduction) broadcast to all 128
    partitions; gamma folding happens on the weights (device-side, once).
  - q^T,k^T computed as [o, n] (fp32r matmuls); v as [n, o] (pixel-partition).
  - Attention runs transposed: sim_T[j, i] = k_T^T q_T with j (kv index) on
    psum partitions, i free. K is zero-padded to 128 (K=64 matmuls are 2x
    slower on PE and row-tiling is pathological). exp on ACT at [128, 1024]
    grain. av matmul: lhsT = [v | 1] (j, 65) bf16 -> out_T [65, i]; row 64
    accumulates the softmax denominator. Normalization via DVE reciprocal +
    DMA partition-broadcast + DVE multiply into attnout [c', n] fp32r.
  - mem_kv + padding live in a 9th j-chunk: k columns are zero there except
    the 4 mem rows, and vext rows beyond the mem rows are zero, so the padded
    exp(0)=1 entries contribute nothing to either the numerator or denominator.
"""

import numpy as np

import concourse.bass as bass
import concourse.mybir as mybir
import concourse.tile as tile
from concourse import bacc
from concourse.bass_utils import run_bass_kernel_spmd

F32 = mybir.dt.float32
F32R = mybir.dt.float32r
BF16 = mybir.dt.bfloat16
AF = mybir.ActivationFunctionType

NCORES = 8
B = 16
C = 512
N = 1024          # pixels = 32*32
HEADS = 8
DH = 64
NMEM = 4
PB = B // NCORES  # batch elements per core
CT = C // 128     # channel partition-tiles
JC = 9            # j chunks: 8 pixel chunks + 1 (mem + zero pad)
VW = HEADS * (DH + 1)  # vext width: per head [v | ones] = 65


def _build():
    nc = bacc.Bacc()
    x_ext = nc.declare_dram_parameter("x", [PB, C, N], F32, isOutput=False)
    wqkvt_ext = nc.declare_dram_parameter("wqkvt", [C, 3 * C], F32, isOutput=False)
    wot_ext = nc.declare_dram_parameter("wot", [C, C], F32, isOutput=False)
    gammat_ext = nc.declare_dram_parameter("gammat", [128, CT], F32, isOutput=False)
    memk_ext = nc.declare_dram_parameter("memk", [128, HEADS, 128], F32, isOutput=False)
    memv_ext = nc.declare_dram_parameter("memv", [128, VW], F32, isOutput=False)
    out_ext = nc.declare_dram_parameter("out", [PB, C, N], F32, isOutput=True)

    with tile.TileContext(nc) as tc:
        with (
            tc.tile_pool(name="const", bufs=1) as const,
            tc.tile_pool(name="wstage", bufs=1) as wstage,
            tc.tile_pool(name="xp", bufs=2) as xp,
            tc.tile_pool(name="data", bufs=1) as data,
            tc.tile_pool(name="qp", bufs=2) as qp,
            tc.tile_pool(name="pp", bufs=4) as pp,
            tc.tile_pool(name="avs", bufs=2) as avsp,
            tc.tile_pool(name="rp", bufs=2) as rp,
            tc.tile_pool(name="ob", bufs=2) as obp,
            tc.tile_pool(name="qkv_ps", bufs=2, space="PSUM") as qkv_ps,
            tc.tile_pool(name="sim_ps", bufs=2, space="PSUM") as sim_ps,
            tc.tile_pool(name="av_ps", bufs=2, space="PSUM") as av_ps,
        ):
            # ---------------- x loads first (don't queue behind weight DMAs) ----
            xraws = []
            for bb in range(PB):
                xr = xp.tile([128, CT, N], F32, tag="xraw")
                for t in range(CT):
                    nc.sync.dma_start(out=xr[:, t, :], in_=x_ext[bb, t * 128:(t + 1) * 128, :])
                xraws.append(xr)

            # ---------------- per-core constants ----------------
            wqkv = const.tile([128, CT, 3 * C], F32R, tag="wqkv")
            wo = const.tile([128, CT, C], F32R, tag="wo")
            g1 = const.tile([128, CT], F32, tag="g1")
            g1q = const.tile([128, CT], F32, tag="g1q")
            ones128 = const.tile([128, 128], BF16, tag="ones128")
            ones1 = const.tile([128, 64], F32R, tag="ones1")
            kTp = const.tile([128, HEADS, 128 * JC], BF16, tag="kTp")
            vext = const.tile([128, JC, VW], BF16, tag="vext")

            gsb = const.tile([128, CT], F32, tag="gsb")
            nc.scalar.dma_start(out=gsb, in_=gammat_ext[:, :])
            nc.scalar.activation(out=g1, in_=gsb, func=AF.Copy, bias=1.0)
            nc.scalar.activation(out=g1q, in_=gsb, func=AF.Copy, bias=1.0, scale=1.0)
            nc.scalar.mul(out=g1q, in_=g1q, mul=DH ** -0.5)

            nc.vector.memset(ones128, 1.0)
            nc.vector.memset(ones1.bitcast(F32), 1.0)
            nc.vector.memset(kTp, 0.0)
            nc.vector.memset(vext, 0.0)

            # weights: DMA f32 staging, scale by (gamma+1) [and 1/sqrt(dh) for q]
            for t in range(CT):
                ws = wstage.tile([128, 3 * C], F32, tag="ws")
                nc.scalar.dma_start(out=ws, in_=wqkvt_ext[t * 128:(t + 1) * 128, :])
                nc.scalar.activation(out=wqkv[:, t, 0:C], in_=ws[:, 0:C],
                                     func=AF.Copy, scale=g1q[:, t:t + 1])
                nc.scalar.activation(out=wqkv[:, t, C:3 * C], in_=ws[:, C:3 * C],
                                     func=AF.Copy, scale=g1[:, t:t + 1])
            for t in range(CT):
                ws = wstage.tile([128, 3 * C], F32, tag="ws")
                nc.scalar.dma_start(out=ws[:, 0:C], in_=wot_ext[t * 128:(t + 1) * 128, :])
                nc.vector.tensor_copy(out=wo[:, t, :], in_=ws[:, 0:C])

            # mem_kv constants -> bf16 tiles (9th j-chunk)
            ws = wstage.tile([128, 3 * C], F32, tag="ws")
            nc.scalar.dma_start(out=ws[:, 0:HEADS * 128],
                              in_=memk_ext[:, :, :].rearrange("p h c -> p (h c)"))
            nc.vector.tensor_copy(
                out=kTp[:, :, 8 * 128:9 * 128],
                in_=ws[:, 0:HEADS * 128].rearrange("p (h c) -> p h c", c=128))
            ws2 = wstage.tile([128, 3 * C], F32, tag="ws")
            nc.scalar.dma_start(out=ws2[:, 0:VW], in_=memv_ext[:, :])
            nc.vector.tensor_copy(out=vext[:, 8, :], in_=ws2[:, 0:VW])
            # ones column for the pixel j-chunks (col 64 of each head slot)
            ones_cols = vext[:, 0:8, :].rearrange("p j (h c) -> p j h c", c=DH + 1)[:, :, :, DH:DH + 1]
            nc.vector.memset(ones_cols, 1.0)

            # ---------------- per-batch-element pipeline ----------------
            for bb in range(PB):
                # RMSNorm scale (per-pixel, broadcast on partitions)
                xraw = xraws[bb]
                xsq = data.tile([128, CT, N], BF16, tag="xsq")
                for t in range(CT):
                    nc.vector.tensor_mul(out=xsq[:, t, :], in0=xraw[:, t, :], in1=xraw[:, t, :])
                ss = sim_ps.tile([128, N], F32, tag="sim")
                for h2 in range(2):
                    for t in range(CT):
                        nc.tensor.matmul(ss[:, h2 * 512:(h2 + 1) * 512], ones128,
                                         xsq[:, t, h2 * 512:(h2 + 1) * 512],
                                         start=(t == 0), stop=(t == CT - 1))
                sroot = data.tile([128, N], F32, tag="sroot")
                nc.scalar.activation(out=sroot, in_=ss, func=AF.Sqrt, scale=1.0 / C)
                snorm = data.tile([128, N], F32, tag="snorm")
                nc.vector.reciprocal_approx_fast(out=snorm, in_=sroot)
                xn = data.tile([128, CT, N], F32R, tag="xn")
                for t in range(CT):
                    nc.vector.tensor_mul(out=xn[:, t, :], in0=xraw[:, t, :], in1=snorm)

                # q^T, k^T projections: [o, n] (fp32r)
                qT = qp.tile([128, CT, N], BF16, tag="qT")
                for mc in range(8):  # o-chunks: 0-3 q, 4-7 k
                    for h2 in range(2):
                        ps = qkv_ps.tile([128, 512], F32, tag="q")
                        for t in range(CT):
                            nc.tensor.matmul(ps, wqkv[:, t, mc * 128:(mc + 1) * 128],
                                             xn[:, t, h2 * 512:(h2 + 1) * 512],
                                             start=(t == 0), stop=(t == CT - 1))
                        if mc < 4:
                            nc.vector.tensor_copy(out=qT[:, mc, h2 * 512:(h2 + 1) * 512], in_=ps)
                        else:
                            h0, h1 = 2 * (mc - 4), 2 * (mc - 4) + 1
                            nc.vector.tensor_copy(
                                out=kTp[0:64, h0, h2 * 512:(h2 + 1) * 512], in_=ps[0:64, :])
                            nc.vector.tensor_copy(
                                out=kTp[64:128, h1, h2 * 512:(h2 + 1) * 512], in_=ps[64:128, :])

                # v projection: [n, o_v] (pixel partitions) -> vext (bf16, strided)
                for ic in range(8):
                    ps = qkv_ps.tile([128, 512], F32, tag="q")
                    for t in range(CT):
                        nc.tensor.matmul(ps, xn[:, t, ic * 128:(ic + 1) * 128],
                                         wqkv[:, t, 2 * C:3 * C],
                                         start=(t == 0), stop=(t == CT - 1))
                    ps_h = ps[:, :].rearrange("p (h c) -> p h c", c=DH)
                    vdst = vext[:, ic, :].rearrange("p (h c) -> p h c", c=DH + 1)[:, :, 0:DH]
                    nc.vector.tensor_copy(out=vdst, in_=ps_h)

                # attention per head
                attn = data.tile([128, CT, N], F32R, tag="attn")
                for h in range(HEADS):
                    av0 = av_ps.tile([65, 512], F32, tag="av")
                    av1 = av_ps.tile([65, 512], F32, tag="av")
                    avt = (av0, av1)
                    for jc in range(JC):
                        st = sim_ps.tile([128, N], F32, tag="sim")
                        for h2 in range(2):
                            nc.tensor.matmul(st[:, h2 * 512:(h2 + 1) * 512],
                                             kTp[:, h, jc * 128:(jc + 1) * 128],
                                             qT[:, h // 2, h2 * 512:(h2 + 1) * 512],
                                             start=True, stop=True)
                        p = pp.tile([128, N], BF16, tag="p")
                        nc.scalar.activation(out=p, in_=st, func=AF.Exp)
                        for h2 in range(2):
                            nc.tensor.matmul(avt[h2], vext[:, jc, h * (DH + 1):(h + 1) * (DH + 1)],
                                             p[:, h2 * 512:(h2 + 1) * 512],
                                             start=(jc == 0), stop=(jc == JC - 1))
                    for h2 in range(2):
                        avb = avsp.tile([65, 512], F32R, tag="avs")
                        nc.vector.tensor_copy(out=avb, in_=avt[h2])
                        bc = av_ps.tile([64, 512], F32, tag="av")
                        nc.tensor.matmul(bc, ones1[64:65, :], avb[64:65, :], start=True, stop=True)
                        rcp = rp.tile([64, 512], F32, tag="rcp")
                        nc.vector.reciprocal_approx_fast(out=rcp, in_=bc)
                        nc.vector.tensor_mul(
                            out=attn[64 * (h % 2):64 * (h % 2) + 64, h // 2,
                                     h2 * 512:(h2 + 1) * 512],
                            in0=avb[0:64, :].bitcast(F32), in1=rcp)

                # output projection [o, n] fp32r and store
                for mc in range(CT):
                    ob = obp.tile([128, N], F32, tag="ob")
                    for h2 in range(2):
                        ps = qkv_ps.tile([128, 512], F32, tag="q")
                        for t in range(CT):
                            nc.tensor.matmul(ps, wo[:, t, mc * 128:(mc + 1) * 128],
                                             attn[:, t, h2 * 512:(h2 + 1) * 512],
                                             start=(t == 0), stop=(t == CT - 1))
                        nc.vector.tensor_copy(out=ob[:, h2 * 512:(h2 + 1) * 512], in_=ps)
                    nc.sync.dma_start(out=out_ext[bb, mc * 128:(mc + 1) * 128, :], in_=ob)
    nc.compile()
    return nc


_NC_CACHE = []


def kernel(x, gamma, mem_kv, w_qkv, w_out, _trace=False):
    x = np.asarray(x, dtype=np.float32)
    gamma = np.asarray(gamma, dtype=np.float32)
    mem_kv = np.asarray(mem_kv, dtype=np.float32)
    w_qkv = np.asarray(w_qkv, dtype=np.float32)
    w_out = np.asarray(w_out, dtype=np.float32)

    b, c, hh, ww = x.shape
    n = hh * ww
    xs = x.reshape(b, c, n)

    wqkvt = np.ascontiguousarray(w_qkv.T)          # [c, 3c]
    wot = np.ascontiguousarray(w_out.T)            # [c, c]
    gammat = np.ascontiguousarray(gamma.reshape(CT, 128).T)  # [128, CT]

    memk = np.zeros((128, HEADS, 128), np.float32)
    memv = np.zeros((128, VW), np.float32)
    for h in range(HEADS):
        r0 = 64 * (h % 2)
        memk[r0:r0 + DH, h, 0:NMEM] = mem_kv[0, h].T      # [dh, nmem]
        memv[0:NMEM, h * (DH + 1):h * (DH + 1) + DH] = mem_kv[1, h]
        memv[0:NMEM, h * (DH + 1) + DH] = 1.0

    if not _NC_CACHE:
        _NC_CACHE.append(_build())
    nc = _NC_CACHE[0]

    in_maps = []
    for core in range(NCORES):
        in_maps.append({
            "x": np.ascontiguousarray(xs[core * PB:(core + 1) * PB]),
            "wqkvt": wqkvt,
            "wot": wot,
            "gammat": gammat,
            "memk": memk,
            "memv": memv,
        })
    res = run_bass_kernel_spmd(nc, in_maps, core_ids=list(range(NCORES)), trace=_trace)
    out = np.concatenate([res.results[core]["out"] for core in range(NCORES)], axis=0)
    kernel.last_result = res
    return out.reshape(b, c, hh, ww)


# revision 13
# speedup vs baseline: 1.0336x; 1.0323x over previous
"""Trainium2 Bass kernel for nn_Attention_7945689497706.

Reference math (per batch element b of 16):
  xn = RMSNorm(x) = x / ||x||_c * (gamma+1) * sqrt(512)        x: [512, 32*32]
  qkv = w_qkv @ xn ; split q,k,v [512, 1024] each; heads of 64
  q *= 64^-0.5 ; k,v get 4 mem_kv rows prepended (shared across batch)
  out = softmax(q k^T) v per head -> w_out @ out

Distribution: data-parallel over batch, 2 batch elements per core, weights
replicated. No collectives.

Kernel layout strategy (per core):
  - x kept [c, n] (c on partitions, 4 tiles of 128). Per-pixel inverse norm is
    computed with a ones-matmul (partition reduction) broadcast to all 128
    partitions; gamma folding happens on the weights (device-side, once).
  - q^T,k^T computed as [o, n] (fp32r matmuls); v as [n, o] (pixel-partition).
  - Attention runs transposed: sim_T[j, i] = k_T^T q_T with j (kv index) on
    psum partitions, i free. K is zero-padded to 128 (K=64 matmuls are 2x
    slower on PE and row-tiling is pathological). exp on ACT at [128, 1024]
    grain. av matmul: lhsT = [v | 1] (j, 65) bf16 -> out_T [65, i]; row 64
    accumulates the softmax denominator. Normalization via DVE reciprocal +
    DMA partition-broadcast + DVE multiply into attnout [c', n] fp32r.
  - mem_kv + padding live in a 9th j-chunk: k columns are zero there except
    the 4 mem rows, and vext rows beyond the mem rows are zero, so the padded
    exp(0)=1 entries contribute nothing to either the numerator or denominator.
"""

import numpy as np

import concourse.bass as bass
import concourse.mybir as mybir
import concourse.tile as tile
from concourse import bacc
from concourse.bass_utils import run_bass_kernel_spmd

F32 = mybir.dt.float32
F32R = mybir.dt.float32r
BF16 = mybir.dt.bfloat16
AF = mybir.ActivationFunctionType

NCORES = 8
B = 16
C = 512
N = 1024          # pixels = 32*32
HEADS = 8
DH = 64
NMEM = 4
PB = B // NCORES  # batch elements per core
CT = C // 128     # channel partition-tiles
JC = 9            # j chunks: 8 pixel chunks + 1 (mem + zero pad)
VW = HEADS * (DH + 1)  # vext width: per head [v | ones] = 65


def _build():
    nc = bacc.Bacc()
    x_ext = nc.declare_dram_parameter("x", [PB, C, N], F32, isOutput=False)
    wqkvt_ext = nc.declare_dram_parameter("wqkvt", [C, 3 * C], F32, isOutput=False)
    wot_ext = nc.declare_dram_parameter("wot", [C, C], F32, isOutput=False)
    gammat_ext = nc.declare_dram_parameter("gammat", [128, CT], F32, isOutput=False)
    memk_ext = nc.declare_dram_parameter("memk", [128, HEADS, 128], F32, isOutput=False)
    memv_ext = nc.declare_dram_parameter("memv", [128, VW], F32, isOutput=False)
    out_ext = nc.declare_dram_parameter("out", [PB, C, N], F32, isOutput=True)

    with tile.TileContext(nc) as tc:
        with (
            tc.tile_pool(name="const", bufs=1) as const,
            tc.tile_pool(name="wstage", bufs=1) as wstage,
            tc.tile_pool(name="xp", bufs=2) as xp,
            tc.tile_pool(name="data", bufs=1) as data,
            tc.tile_pool(name="qp", bufs=2) as qp,
            tc.tile_pool(name="pp", bufs=4) as pp,
            tc.tile_pool(name="avs", bufs=2) as avsp,
            tc.tile_pool(name="rp", bufs=2) as rp,
            tc.tile_pool(name="ob", bufs=2) as obp,
            tc.tile_pool(name="qkv_ps", bufs=2, space="PSUM") as qkv_ps,
            tc.tile_pool(name="sim_ps", bufs=2, space="PSUM") as sim_ps,
            tc.tile_pool(name="av_ps", bufs=2, space="PSUM") as av_ps,
        ):
            # ---------------- x loads first (don't queue behind weight DMAs) ----
            xraws = []
            for bb in range(PB):
                xr = xp.tile([128, CT, N], F32, tag="xraw")
                for t in range(CT):
                    nc.sync.dma_start(out=xr[:, t, :], in_=x_ext[bb, t * 128:(t + 1) * 128, :])
                xraws.append(xr)

            # ---------------- per-core constants ----------------
            wqkv = const.tile([128, CT, 3 * C], F32R, tag="wqkv")
            wo = const.tile([128, CT, C], F32R, tag="wo")
            g1 = const.tile([128, CT], F32, tag="g1")
            g1q = const.tile([128, CT], F32, tag="g1q")
            ones128 = const.tile([128, 128], BF16, tag="ones128")
            ones1 = const.tile([128, 64], F32R, tag="ones1")
            kTp = const.tile([128, HEADS, 128 * JC], BF16, tag="kTp")
            vext = const.tile([128, JC, VW], BF16, tag="vext")

            gsb = const.tile([128, CT], F32, tag="gsb")
            nc.sync.dma_start(out=gsb, in_=gammat_ext[:, :])
            nc.scalar.activation(out=g1, in_=gsb, func=AF.Copy, bias=1.0)
            nc.scalar.activation(out=g1q, in_=gsb, func=AF.Copy, bias=1.0, scale=1.0)
            nc.scalar.mul(out=g1q, in_=g1q, mul=DH ** -0.5)

            nc.vector.memset(ones128, 1.0)
            nc.vector.memset(ones1.bitcast(F32), 1.0)
            nc.vector.memset(kTp, 0.0)
            nc.vector.memset(vext, 0.0)

            # weights: DMA f32 staging, scale by (gamma+1) [and 1/sqrt(dh) for q]
            for t in range(CT):
                ws = wstage.tile([128, 3 * C], F32, tag="ws")
                nc.sync.dma_start(out=ws, in_=wqkvt_ext[t * 128:(t + 1) * 128, :])
                nc.scalar.activation(out=wqkv[:, t, 0:C], in_=ws[:, 0:C],
                                     func=AF.Copy, scale=g1q[:, t:t + 1])
                nc.scalar.activation(out=wqkv[:, t, C:3 * C], in_=ws[:, C:3 * C],
                                     func=AF.Copy, scale=g1[:, t:t + 1])
            for t in range(CT):
                ws = wstage.tile([128, 3 * C], F32, tag="ws")
                nc.sync.dma_start(out=ws[:, 0:C], in_=wot_ext[t * 128:(t + 1) * 128, :])
                nc.vector.tensor_copy(out=wo[:, t, :], in_=ws[:, 0:C])

            # mem_kv constants -> bf16 tiles (9th j-chunk)
            ws = wstage.tile([128, 3 * C], F32, tag="ws")
            nc.sync.dma_start(out=ws[:, 0:HEADS * 128],
                              in_=memk_ext[:, :, :].rearrange("p h c -> p (h c)"))
            nc.vector.tensor_copy(
                out=kTp[:, :, 8 * 128:9 * 128],
                in_=ws[:, 0:HEADS * 128].rearrange("p (h c) -> p h c", c=128))
            ws2 = wstage.tile([128, 3 * C], F32, tag="ws")
            nc.sync.dma_start(out=ws2[:, 0:VW], in_=memv_ext[:, :])
            nc.vector.tensor_copy(out=vext[:, 8, :], in_=ws2[:, 0:VW])
            # ones column for the pixel j-chunks (col 64 of each head slot)
            ones_cols = vext[:, 0:8, :].rearrange("p j (h c) -> p j h c", c=DH + 1)[:, :, :, DH:DH + 1]
            nc.vector.memset(ones_cols, 1.0)

            # ---------------- per-batch-element pipeline ----------------
            for bb in range(PB):
                # RMSNorm scale (per-pixel, broadcast on partitions)
                xraw = xraws[bb]
                xsq = data.tile([128, CT, N], BF16, tag="xsq")
                for t in range(CT):
                    nc.vector.tensor_mul(out=xsq[:, t, :], in0=xraw[:, t, :], in1=xraw[:, t, :])
                ss = sim_ps.tile([128, N], F32, tag="sim")
                for h2 in range(2):
                    for t in range(CT):
                        nc.tensor.matmul(ss[:, h2 * 512:(h2 + 1) * 512], ones128,
                                         xsq[:, t, h2 * 512:(h2 + 1) * 512],
                                         start=(t == 0), stop=(t == CT - 1))
                sroot = data.tile([128, N], F32, tag="sroot")
                nc.scalar.activation(out=sroot, in_=ss, func=AF.Sqrt, scale=1.0 / C)
                snorm = data.tile([128, N], F32, tag="snorm")
                nc.vector.reciprocal_approx_fast(out=snorm, in_=sroot)
                xn = data.tile([128, CT, N], F32R, tag="xn")
                for t in range(CT):
                    nc.vector.tensor_mul(out=xn[:, t, :], in0=xraw[:, t, :], in1=snorm)

                # q^T, k^T projections: [o, n] (fp32r)
                qT = qp.tile([128, CT, N], BF16, tag="qT")
                for mc in range(8):  # o-chunks: 0-3 q, 4-7 k
                    for h2 in range(2):
                        ps = qkv_ps.tile([128, 512], F32, tag="q")
                        for t in range(CT):
                            nc.tensor.matmul(ps, wqkv[:, t, mc * 128:(mc + 1) * 128],
                                             xn[:, t, h2 * 512:(h2 + 1) * 512],
                                             start=(t == 0), stop=(t == CT - 1))
                        if mc < 4:
                            nc.vector.tensor_copy(out=qT[:, mc, h2 * 512:(h2 + 1) * 512], in_=ps)
                        else:
                            h0, h1 = 2 * (mc - 4), 2 * (mc - 4) + 1
                            nc.vector.tensor_copy(
                                out=kTp[0:64, h0, h2 * 512:(h2 + 1) * 512], in_=ps[0:64, :])
                            nc.vector.tensor_copy(
                                out=kTp[64:128, h1, h2 * 512:(h2 + 1) * 512], in_=ps[64:128, :])

                # v projection: [n, o_v] (pixel partitions) -> vext (bf16, strided)
                for ic in range(8):
                    ps = qkv_ps.tile([128, 512], F32, tag="q")
                    for t in range(CT):
                        nc.tensor.matmul(ps, xn[:, t, ic * 128:(ic + 1) * 128],
                                         wqkv[:, t, 2 * C:3 * C],
                                         start=(t == 0), stop=(t == CT - 1))
                    ps_h = ps[:, :].rearrange("p (h c) -> p h c", c=DH)
                    vdst = vext[:, ic, :].rearrange("p (h c) -> p h c", c=DH + 1)[:, :, 0:DH]
                    nc.vector.tensor_copy(out=vdst, in_=ps_h)

                # attention per head
                attn = data.tile([128, CT, N], F32R, tag="attn")
                for h in range(HEADS):
                    av0 = av_ps.tile([65, 512], F32, tag="av")
                    av1 = av_ps.tile([65, 512], F32, tag="av")
                    avt = (av0, av1)
                    for jc in range(JC):
                        st = sim_ps.tile([128, N], F32, tag="sim")
                        for h2 in range(2):
                            nc.tensor.matmul(st[:, h2 * 512:(h2 + 1) * 512],
                                             kTp[:, h, jc * 128:(jc + 1) * 128],
                                             qT[:, h // 2, h2 * 512:(h2 + 1) * 512],
                                             start=True, stop=True)
                        p = pp.tile([128, N], BF16, tag="p")
                        nc.scalar.activation(out=p, in_=st, func=AF.Exp)
                        for h2 in range(2):
                            nc.tensor.matmul(avt[h2], vext[:, jc, h * (DH + 1):(h + 1) * (DH + 1)],
                                             p[:, h2 * 512:(h2 + 1) * 512],
                                             start=(jc == 0), stop=(jc == JC - 1))
                    for h2 in range(2):
                        avb = avsp.tile([65, 512], F32R, tag="avs")
                        nc.vector.tensor_copy(out=avb, in_=avt[h2])
                        bc = av_ps.tile([64, 512], F32, tag="av")
                        nc.tensor.matmul(bc, ones1[64:65, :], avb[64:65, :], start=True, stop=True)
                        rcp = rp.tile([64, 512], F32, tag="rcp")
                        nc.vector.reciprocal_approx_fast(out=rcp, in_=bc)
                        nc.vector.tensor_mul(
                            out=attn[64 * (h % 2):64 * (h % 2) + 64, h // 2,
                                     h2 * 512:(h2 + 1) * 512],
                            in0=avb[0:64, :].bitcast(F32), in1=rcp)

                # output projection [o, n] fp32r and store
                for mc in range(CT):
                    ob = obp.tile([128, N], F32, tag="ob")
                    for h2 in range(2):
                        ps = qkv_ps.tile([128, 512], F32, tag="q")
                        for t in range(CT):
                            nc.tensor.matmul(ps, wo[:, t, mc * 128:(mc + 1) * 128],
                                             attn[:, t, h2 * 512:(h2 + 1) * 512],
                                             start=(t == 0), stop=(t == CT - 1))
                        nc.vector.tensor_copy(out=ob[:, h2 * 512:(h2 + 1) * 512], in_=ps)
                    nc.sync.dma_start(out=out_ext[bb, mc * 128:(mc + 1) * 128, :], in_=ob)
    nc.compile()
    return nc


_NC_CACHE = []


def kernel(x, gamma, mem_kv, w_qkv, w_out, _trace=False):
    x = np.asarray(x, dtype=np.float32)
    gamma = np.asarray(gamma, dtype=np.float32)
    mem_kv = np.asarray(mem_kv, dtype=np.float32)
    w_qkv = np.asarray(w_qkv, dtype=np.float32)
    w_out = np.asarray(w_out, dtype=np.float32)

    b, c, hh, ww = x.shape
    n = hh * ww
    xs = x.reshape(b, c, n)

    wqkvt = np.ascontiguousarray(w_qkv.T)          # [c, 3c]
    wot = np.ascontiguousarray(w_out.T)            # [c, c]
    gammat = np.ascontiguousarray(gamma.reshape(CT, 128).T)  # [128, CT]

    memk = np.zeros((128, HEADS, 128), np.float32)
    memv = np.zeros((128, VW), np.float32)
    for h in range(HEADS):
        r0 = 64 * (h % 2)
        memk[r0:r0 + DH, h, 0:NMEM] = mem_kv[0, h].T      # [dh, nmem]
        memv[0:NMEM, h * (DH + 1):h * (DH + 1) + DH] = mem_kv[1, h]
        memv[0:NMEM, h * (DH + 1) + DH] = 1.0

    if not _NC_CACHE:
        _NC_CACHE.append(_build())
    nc = _NC_CACHE[0]

    in_maps = []
    for core in range(NCORES):
        in_maps.append({
            "x": np.ascontiguousarray(xs[core * PB:(core + 1) * PB]),
            "wqkvt": wqkvt,
            "wot": wot,
            "gammat": gammat,
            "memk": memk,
            "memv": memv,
        })
    res = run_bass_kernel_spmd(nc, in_maps, core_ids=list(range(NCORES)), trace=_trace)
    out = np.concatenate([res.results[core]["out"] for core in range(NCORES)], axis=0)
    kernel.last_result = res
    return out.reshape(b, c, hh, ww)


# revision 16
# speedup vs baseline: 1.0905x; 1.0551x over previous
"""Trainium2 Bass kernel for nn_Attention_7945689497706.

Distribution: data-parallel over batch, 2 batch elements per core, weights
replicated, no collectives.

Per-core layout:
  - RMSNorm via ones-matmul partition reduction, gamma folded into weights.
  - q^T,k^T in [o, n] fp32r; v in [n, o] feeding a bf16 [v|1] (j, 65) tile.
  - Attention transposed (j on psum partitions): sim_T = kTpad^T qT with K
    zero-padded to 128; exp on ACT at [128,1024] grain; av lhsT = vext so the
    ones column accumulates softmax denominators; normalization = K=1 matmul
    broadcast + DVE fast-reciprocal + multiply.
  - mem_kv + padding in a 9th j-chunk (zero k-cols / zero v-rows make the
    padded lanes contribute nothing).
  - The two batch elements are software-pipelined: batch 1's norm/projections
    are emitted inside batch 0's attention loop (per-head kTp handoff) so the
    PE fills the ACT-bound exp bubbles.
"""

import numpy as np

import concourse.bass as bass
import concourse.mybir as mybir
import concourse.tile as tile
from concourse import bacc
from concourse.bass_utils import run_bass_kernel_spmd

F32 = mybir.dt.float32
F32R = mybir.dt.float32r
BF16 = mybir.dt.bfloat16
AF = mybir.ActivationFunctionType

NCORES = 8
B = 16
C = 512
N = 1024          # pixels = 32*32
HEADS = 8
DH = 64
NMEM = 4
PB = B // NCORES  # batch elements per core
CT = C // 128     # channel partition-tiles
JC = 9            # j chunks: 8 pixel chunks + 1 (mem + zero pad)
VW = HEADS * (DH + 1)  # vext width: per head [v | ones] = 65


def _build():
    nc = bacc.Bacc()
    x_ext = nc.declare_dram_parameter("x", [PB, C, N], F32, isOutput=False)
    wqkvt_ext = nc.declare_dram_parameter("wqkvt", [C, 3 * C], F32, isOutput=False)
    wot_ext = nc.declare_dram_parameter("wot", [C, C], F32, isOutput=False)
    gammat_ext = nc.declare_dram_parameter("gammat", [128, CT], F32, isOutput=False)
    memk_ext = nc.declare_dram_parameter("memk", [128, HEADS, 128], F32, isOutput=False)
    memv_ext = nc.declare_dram_parameter("memv", [128, VW], F32, isOutput=False)
    out_ext = nc.declare_dram_parameter("out", [PB, C, N], F32, isOutput=True)

    with tile.TileContext(nc) as tc:
        with (
            tc.tile_pool(name="const", bufs=1) as const,
            tc.tile_pool(name="wstage", bufs=1) as wstage,
            tc.tile_pool(name="xp", bufs=2) as xp,
            tc.tile_pool(name="data", bufs=1) as data,
            tc.tile_pool(name="qp", bufs=2) as qp,
            tc.tile_pool(name="pp", bufs=4) as pp,
            tc.tile_pool(name="avs", bufs=2) as avsp,
            tc.tile_pool(name="rp", bufs=2) as rp,
            tc.tile_pool(name="ob", bufs=2) as obp,
            tc.tile_pool(name="qkv_ps", bufs=2, space="PSUM") as qkv_ps,
            tc.tile_pool(name="sim_ps", bufs=2, space="PSUM") as sim_ps,
            tc.tile_pool(name="av_ps", bufs=2, space="PSUM") as av_ps,
        ):
            # ------------ x loads first (don't queue behind weight DMAs) -------
            xraws = []
            for bb in range(PB):
                xr = xp.tile([128, CT, N], F32, tag="xraw")
                for t in range(CT):
                    nc.sync.dma_start(out=xr[:, t, :], in_=x_ext[bb, t * 128:(t + 1) * 128, :])
                xraws.append(xr)

            # ---------------- per-core constants ----------------
            wqkv = const.tile([128, CT, 3 * C], F32R, tag="wqkv")
            wo = const.tile([128, CT, C], F32R, tag="wo")
            g1 = const.tile([128, CT], F32, tag="g1")
            g1q = const.tile([128, CT], F32, tag="g1q")
            ones128 = const.tile([128, 128], BF16, tag="ones128")
            ones1 = const.tile([128, 64], F32R, tag="ones1")
            kTp = const.tile([128, HEADS, 128 * JC], BF16, tag="kTp")
            vextA = const.tile([128, JC, VW], BF16, tag="vextA")
            vextB = const.tile([128, JC, VW], BF16, tag="vextB")
            vexts = [vextA, vextB]

            gsb = const.tile([128, CT], F32, tag="gsb")
            nc.sync.dma_start(out=gsb, in_=gammat_ext[:, :])
            nc.scalar.activation(out=g1, in_=gsb, func=AF.Copy, bias=1.0)
            nc.scalar.activation(out=g1q, in_=gsb, func=AF.Copy, bias=1.0, scale=1.0)
            nc.scalar.mul(out=g1q, in_=g1q, mul=DH ** -0.5)

            nc.vector.memset(ones128, 1.0)
            nc.vector.memset(ones1.bitcast(F32), 1.0)
            nc.vector.memset(kTp, 0.0)

            # weights: DMA f32 staging, scale by (gamma+1) [and 1/sqrt(dh) for q]
            for t in range(CT):
                ws = wstage.tile([128, 3 * C], F32, tag="ws")
                nc.sync.dma_start(out=ws, in_=wqkvt_ext[t * 128:(t + 1) * 128, :])
                nc.scalar.activation(out=wqkv[:, t, 0:C], in_=ws[:, 0:C],
                                     func=AF.Copy, scale=g1q[:, t:t + 1])
                nc.scalar.activation(out=wqkv[:, t, C:3 * C], in_=ws[:, C:3 * C],
                                     func=AF.Copy, scale=g1[:, t:t + 1])
            for t in range(CT):
                ws = wstage.tile([128, 3 * C], F32, tag="ws")
                nc.sync.dma_start(out=ws[:, 0:C], in_=wot_ext[t * 128:(t + 1) * 128, :])
                nc.vector.tensor_copy(out=wo[:, t, :], in_=ws[:, 0:C])

            # mem_kv constants -> bf16 tiles (9th j-chunk)
            ws = wstage.tile([128, 3 * C], F32, tag="ws")
            nc.sync.dma_start(out=ws[:, 0:HEADS * 128],
                              in_=memk_ext[:, :, :].rearrange("p h c -> p (h c)"))
            nc.vector.tensor_copy(
                out=kTp[:, :, 8 * 128:9 * 128],
                in_=ws[:, 0:HEADS * 128].rearrange("p (h c) -> p h c", c=128))
            ws2 = wstage.tile([128, 3 * C], F32, tag="ws")
            nc.sync.dma_start(out=ws2[:, 0:VW], in_=memv_ext[:, :])
            for v in vexts:
                nc.vector.memset(v, 0.0)
                nc.vector.tensor_copy(out=v[:, 8, :], in_=ws2[:, 0:VW])
                oc = v[:, 0:8, :].rearrange("p j (h c) -> p j h c", c=DH + 1)[:, :, :, DH:DH + 1]
                nc.vector.memset(oc, 1.0)

            # ---------------- pipeline stages ----------------
            def norm(bb):
                """x -> xn (fp32r, per-pixel normalized)."""
                xraw = xraws[bb]
                xsq = data.tile([128, CT, N], BF16, tag="xsq")
                for t in range(CT):
                    nc.vector.tensor_mul(out=xsq[:, t, :], in0=xraw[:, t, :], in1=xraw[:, t, :])
                ss = sim_ps.tile([128, N], F32, tag="sim")
                for h2 in range(2):
                    for t in range(CT):
                        nc.tensor.matmul(ss[:, h2 * 512:(h2 + 1) * 512], ones128,
                                         xsq[:, t, h2 * 512:(h2 + 1) * 512],
                                         start=(t == 0), stop=(t == CT - 1))
                sroot = data.tile([128, N], F32, tag="sroot")
                nc.scalar.activation(out=sroot, in_=ss, func=AF.Sqrt, scale=1.0 / C)
                snorm = data.tile([128, N], F32, tag="snorm")
                nc.vector.reciprocal_approx_fast(out=snorm, in_=sroot)
                xn = data.tile([128, CT, N], F32R, tag="xn" + str(bb))
                for t in range(CT):
                    nc.vector.tensor_mul(out=xn[:, t, :], in0=xraw[:, t, :], in1=snorm)
                return xn

            def qkproj(xn, qT, mcs):
                """o-chunks mcs of the q/k projection; k goes into kTp (padded)."""
                for mc in mcs:
                    for h2 in range(2):
                        ps = qkv_ps.tile([128, 512], F32, tag="q")
                        for t in range(CT):
                            nc.tensor.matmul(ps, wqkv[:, t, mc * 128:(mc + 1) * 128],
                                             xn[:, t, h2 * 512:(h2 + 1) * 512],
                                             start=(t == 0), stop=(t == CT - 1))
                        if mc < 4:
                            nc.vector.tensor_copy(out=qT[:, mc, h2 * 512:(h2 + 1) * 512], in_=ps)
                        else:
                            h0, h1 = 2 * (mc - 4), 2 * (mc - 4) + 1
                            nc.vector.tensor_copy(
                                out=kTp[0:64, h0, h2 * 512:(h2 + 1) * 512], in_=ps[0:64, :])
                            nc.vector.tensor_copy(
                                out=kTp[64:128, h1, h2 * 512:(h2 + 1) * 512], in_=ps[64:128, :])

            def vproj(xn, vext, ics):
                for ic in ics:
                    ps = qkv_ps.tile([128, 512], F32, tag="q")
                    for t in range(CT):
                        nc.tensor.matmul(ps, xn[:, t, ic * 128:(ic + 1) * 128],
                                         wqkv[:, t, 2 * C:3 * C],
                                         start=(t == 0), stop=(t == CT - 1))
                    ps_h = ps[:, :].rearrange("p (h c) -> p h c", c=DH)
                    vdst = vext[:, ic, :].rearrange("p (h c) -> p h c", c=DH + 1)[:, :, 0:DH]
                    nc.vector.tensor_copy(out=vdst, in_=ps_h)

            def head_attn(h, qT, vext, attn):
                av0 = av_ps.tile([65, 512], F32, tag="av")
                av1 = av_ps.tile([65, 512], F32, tag="av")
                avt = (av0, av1)
                for jc in range(JC):
                    st = sim_ps.tile([128, N], F32, tag="sim")
                    for h2 in range(2):
                        nc.tensor.matmul(st[:, h2 * 512:(h2 + 1) * 512],
                                         kTp[:, h, jc * 128:(jc + 1) * 128],
                                         qT[:, h // 2, h2 * 512:(h2 + 1) * 512],
                                         start=True, stop=True)
                    p = pp.tile([128, N], BF16, tag="p")
                    nc.scalar.activation(out=p, in_=st, func=AF.Exp)
                    for h2 in range(2):
                        nc.tensor.matmul(avt[h2], vext[:, jc, h * (DH + 1):(h + 1) * (DH + 1)],
                                         p[:, h2 * 512:(h2 + 1) * 512],
                                         start=(jc == 0), stop=(jc == JC - 1))
                for h2 in range(2):
                    avb = avsp.tile([65, 512], F32R, tag="avs")
                    nc.vector.tensor_copy(out=avb, in_=avt[h2])
                    bc = av_ps.tile([64, 512], F32, tag="av")
                    nc.tensor.matmul(bc, ones1[64:65, :], avb[64:65, :], start=True, stop=True)
                    rcp = rp.tile([64, 512], F32, tag="rcp")
                    nc.vector.reciprocal_approx_fast(out=rcp, in_=bc)
                    nc.vector.tensor_mul(
                        out=attn[64 * (h % 2):64 * (h % 2) + 64, h // 2,
                                 h2 * 512:(h2 + 1) * 512],
                        in0=avb[0:64, :].bitcast(F32), in1=rcp)

            def proj(attn, bb):
                for mc in range(CT):
                    for h2 in range(2):
                        ps = qkv_ps.tile([128, 512], F32, tag="q")
                        for t in range(CT):
                            nc.tensor.matmul(ps, wo[:, t, mc * 128:(mc + 1) * 128],
                                             attn[:, t, h2 * 512:(h2 + 1) * 512],
                                             start=(t == 0), stop=(t == CT - 1))
                        ob = obp.tile([128, 512], F32, tag="ob")
                        nc.vector.tensor_copy(out=ob, in_=ps)
                        nc.sync.dma_start(
                            out=out_ext[bb, mc * 128:(mc + 1) * 128, h2 * 512:(h2 + 1) * 512],
                            in_=ob)

            # ---------------- interleaved schedule ----------------
            xn0 = norm(0)
            qT0 = qp.tile([128, CT, N], BF16, tag="qT")
            qkproj(xn0, qT0, range(8))
            vproj(xn0, vexts[0], range(8))
            xn1 = norm(1)

            qT1 = qp.tile([128, CT, N], BF16, tag="qT")
            attn0 = data.tile([128, CT, N], F32R, tag="attn")
            for h in range(HEADS):
                head_attn(h, qT0, vexts[0], attn0)
                # batch 1 projections fill the exp-bound bubbles; k chunks are
                # written into kTp right after batch 0 finishes reading them.
                qkproj(xn1, qT1, [h // 2] if h % 2 == 0 else [4 + (h - 1) // 2])
                vproj(xn1, vexts[1], [h])
            proj(attn0, 0)

            attn1 = data.tile([128, CT, N], F32R, tag="attn")
            for h in range(HEADS):
                head_attn(h, qT1, vexts[1], attn1)
            proj(attn1, 1)
    nc.compile()
    return nc


_NC_CACHE = []


def kernel(x, gamma, mem_kv, w_qkv, w_out, _trace=False):
    x = np.asarray(x, dtype=np.float32)
    gamma = np.asarray(gamma, dtype=np.float32)
    mem_kv = np.asarray(mem_kv, dtype=np.float32)
    w_qkv = np.asarray(w_qkv, dtype=np.float32)
    w_out = np.asarray(w_out, dtype=np.float32)

    b, c, hh, ww = x.shape
    n = hh * ww
    xs = x.reshape(b, c, n)

    wqkvt = np.ascontiguousarray(w_qkv.T)          # [c, 3c]
    wot = np.ascontiguousarray(w_out.T)            # [c, c]
    gammat = np.ascontiguousarray(gamma.reshape(CT, 128).T)  # [128, CT]

    memk = np.zeros((128, HEADS, 128), np.float32)
    memv = np.zeros((128, VW), np.float32)
    for h in range(HEADS):
        r0 = 64 * (h % 2)
        memk[r0:r0 + DH, h, 0:NMEM] = mem_kv[0, h].T      # [dh, nmem]
        memv[0:NMEM, h * (DH + 1):h * (DH + 1) + DH] = mem_kv[1, h]
        memv[0:NMEM, h * (DH + 1) + DH] = 1.0

    if not _NC_CACHE:
        _NC_CACHE.append(_build())
    nc = _NC_CACHE[0]

    in_maps = []
    for core in range(NCORES):
        in_maps.append({
            "x": np.ascontiguousarray(xs[core * PB:(core + 1) * PB]),
            "wqkvt": wqkvt,
            "wot": wot,
            "gammat": gammat,
            "memk": memk,
            "memv": memv,
        })
    res = run_bass_kernel_spmd(nc, in_maps, core_ids=list(range(NCORES)), trace=_trace)
    out = np.concatenate([res.results[core]["out"] for core in range(NCORES)], axis=0)
    kernel.last_result = res
    return out.reshape(b, c, hh, ww)
